# revision 2
# baseline (speedup 1.0000x reference)
"""CAM (channel attention) module kernel for Trainium2, 8-core data-parallel. v9

Reference computation (per batch b, channel c):
    v = x[b,c]                         # (P=3, HW=4096)
    energy = v @ v.T                   # (3,3) Gram matrix
    att = softmax(rowmax(energy) - energy)
    y[b,c] = gamma * (att @ v) + x[b,c]

Sharding: batch dim (B=8) across the 8 NeuronCores; no cross-core comms.
Per core: channels on SBUF partitions (2 groups of 128), free dim = P*HW.

Design: the per-group period is bound by the DMA floor (26.4 us: 6.3 MB
f32 in + 3.15 MB fp16 out @ 358 GB/s). Everything else is engineered to
hide under it; three co-binding constraints are each addressed:
  1. SBUF port contention (measured: kernel time tracks total SBUF bytes
     once above ~330 KB/partition/group): on-chip compute is fp16 — one
     DVE 2x copy converts each loaded half; PE moving, Gram inputs, and
     the PSUM-drain outputs are all 16-bit. Accum-only Gram ops write
     their throwaway main-out into a 64-element stride-0 region.
  2. DVE busy (TensorScalarPtr has NO fast mode — crosses are 1x): the
     PSUM drains go 11/12 to ACT, softmax exp is one fused pass.
  3. The load-gated tail (last path lands at 17.6 us and gates two cross
     terms + softmax + weights): loads/converts/path-2 crosses are split
     into halves so the chain starts at ~14.8 us and finishes inside the
     DMA window.
  - 3x3 mix on the TensorEngine as accumulating diagonal matmuls
    (diag weights built by one broadcast multiply against an identity
    mask); PSUM f32; fp16 stores (tolerance 2e-2, measured ~2e-4).
  - One-group software pipeline: group g+1's Gram interleaves into the
    engine FIFOs between group g's PSUM drains. Outputs ride gpsimd's
    SWDGE queue, loads ride SP's HWDGE queue.
"""

import numpy as np

import concourse.bacc as bacc
import concourse.mybir as mybir
import concourse.tile as tile
from concourse import masks
from concourse.bass_utils import run_bass_kernel_spmd

B, C, P, H, W = 8, 256, 3, 64, 64
HW = H * W
N_CORES = 8
PARTS = 128

F32 = mybir.dt.float32
F16 = mybir.dt.float16
OUT_DT = mybir.dt.float16
OUT_NP = np.float16
Alu = mybir.AluOpType
Act = mybir.ActivationFunctionType

MM_N = 512      # moving free dim per matmul (psum out = 1 bank)
CONV_N = 1024   # psum->sbuf convert granularity (2 banks, bufs=4)


def build_nc(C_=C, HW_=HW, repeat=1, split_load=True):
    assert C_ % PARTS == 0
    n_groups = C_ // PARTS
    n_conv = HW_ // CONV_N
    mm_per_conv = CONV_N // MM_N
    HALF = HW_ // 2

    nc = bacc.Bacc("TRN2", target_bir_lowering=False, debug=False)

    x_d = nc.dram_tensor("x", [C_, P, HW_], F32, kind="ExternalInput")
    g_d = nc.dram_tensor("gamma", [1, 1], F32, kind="ExternalInput")
    y_d = nc.dram_tensor("y", [C_, P, HW_], OUT_DT, kind="ExternalOutput")

    groups = [t % n_groups for t in range(n_groups * repeat)]
    T = len(groups)

    with tile.TileContext(nc) as tc:
        with (
            tc.tile_pool(name="consts", bufs=1) as consts,
            tc.tile_pool(name="vpool", bufs=2) as vpool,
            tc.tile_pool(name="scratch", bufs=1) as scratch,
            tc.tile_pool(name="ypool", bufs=4) as ypool,
            tc.tile_pool(name="wpool", bufs=2) as wpool,
            tc.tile_pool(name="smalls", bufs=2) as smalls,
            tc.tile_pool(name="psum", bufs=4, space="PSUM") as psum,
        ):
            # --- constants (once) ---
            gsb = consts.tile([1, 1], F32)
            nc.sync.dma_start(gsb[:], g_d[:])
            gamma_bc = consts.tile([PARTS, 1], F32)
            nc.gpsimd.partition_broadcast(gamma_bc[:], gsb[:])

            ident9 = consts.tile([PARTS, 9], F32)
            nc.vector.memset(ident9[:], 0.0)
            for i in range(P):
                nc.vector.memset(ident9[:, 4 * i : 4 * i + 1], 1.0)

            identM = consts.tile([PARTS, 1, PARTS], F32)
            masks.make_identity(nc, identM[:, 0, :])

            def hslice(i, h):
                return slice(i * HW_ + h * HALF, i * HW_ + (h + 1) * HALF)

            def load_half(v32, g, i, h):
                cs = slice(g * PARTS, (g + 1) * PARTS)
                hs = slice(h * HALF, (h + 1) * HALF)
                nc.sync.dma_start(v32[:, i, hs], x_d[cs, i, hs])

            def convert_half(v32, v16, i, h):
                # f32 -> fp16 on DVE (2x_2P: both read ports on one tensor)
                hs = slice(h * HALF, (h + 1) * HALF)
                nc.vector.tensor_copy(v16[:, i, hs], v32[:, i, hs])

            def gout(tag, n):
                # garbage main-out for accum-only ops: a 64-elem region
                # written cyclically (stride-0 outer dim)
                scr = scratch.tile([PARTS, 64], F16, tag=tag, bufs=1,
                                   name="scr")
                return (scr[:].rearrange("p (o f) -> p o f", o=1)
                        .broadcast_to([PARTS, n // 64, 64]))

            def blk(ap):
                return ap.rearrange("p (a b) -> p a b", b=64)

            def diag(v16, E, i):
                nc.scalar.activation(
                    gout("scr_act", HW_), blk(v16[:, i, :]), Act.Square,
                    accum_out=E[:, 4 * i : 4 * i + 1],
                )

            def cross(v16, dst, i, j, h=None):
                """dst[128,1] = sum over (half-)row of v16_i * v16_j on DVE.

                NOTE: tensor_tensor_reduce and gpsimd stt+accum both wedge
                the exec unit on this runtime; DVE scalar_tensor_tensor
                with accum_out is the fused mult+reduce that works.
                """
                sl = slice(None) if h is None else slice(h * HALF,
                                                         (h + 1) * HALF)
                n = HW_ if h is None else HALF
                nc.vector.scalar_tensor_tensor(
                    gout("scr_dve", n), blk(v16[:, i, sl]), 1.0,
                    blk(v16[:, j, sl]),
                    op0=Alu.bypass, op1=Alu.mult,
                    accum_out=dst,
                )

            def softmax_wd(E, Eh):
                """combine half-sums, mirror, softmax(rowmin - E), weights."""
                # cross12 = Eh0+Eh1 -> col5 ; cross02 = Eh2+Eh3 -> col2
                nc.vector.tensor_add(E[:, 5:6], Eh[:, 0:1], Eh[:, 1:2])
                nc.vector.tensor_add(E[:, 2:3], Eh[:, 2:3], Eh[:, 3:4])
                for src, dst in ((1, 3), (5, 7), (2, 6)):
                    nc.scalar.copy(E[:, dst : dst + 1], E[:, src : src + 1])
                E3 = E.rearrange("p (i j) -> p i j", j=P)
                M = smalls.tile([PARTS, P, 1], F32, name="M")
                nc.vector.tensor_reduce(M[:], E3, axis=mybir.AxisListType.X,
                                        op=Alu.min)
                # EN = M - E (one tiny DVE op), then a single fused Exp pass
                EN = smalls.tile([PARTS, P, P], F32, name="EN")
                nc.vector.tensor_sub(EN[:], M[:].broadcast_to([PARTS, P, P]),
                                     E3)
                EX = smalls.tile([PARTS, P, P], F32, name="EX")
                nc.scalar.activation(EX[:], EN[:], Act.Exp)
                S = smalls.tile([PARTS, P, 1], F32, name="S")
                nc.vector.tensor_reduce(S[:], EX[:], axis=mybir.AxisListType.X,
                                        op=Alu.add)
                R = smalls.tile([PARTS, P, 1], F32, name="R")
                nc.vector.reciprocal(R[:], S[:])
                A = smalls.tile([PARTS, P, P], F32, name="A")
                nc.vector.tensor_mul(A[:], EX[:], R[:].broadcast_to([PARTS, P, P]))
                Cf = smalls.tile([PARTS, 9], F32, name="Cf")
                nc.vector.scalar_tensor_tensor(
                    Cf[:].rearrange("p (i j) -> p i j", j=P), A[:],
                    gamma_bc[:, 0:1],
                    ident9[:].rearrange("p (i j) -> p i j", j=P),
                    op0=Alu.mult, op1=Alu.add,
                )
                # fp16 diagonal PE weights (identity part is exact in fp16)
                Wd = wpool.tile([PARTS, 9, PARTS], F16, name="Wd")
                nc.vector.tensor_mul(
                    Wd[:],
                    identM[:].broadcast_to([PARTS, 9, PARTS]),
                    Cf[:].rearrange("p (k o) -> p k o", o=1)
                        .broadcast_to([PARTS, 9, PARTS]),
                )
                return Wd

            def path_compute(g, v16, Wd, i, conv_engine):
                """Mix path i of group g on PE; drain PSUM to fp16; DMA out."""
                cs = slice(g * PARTS, (g + 1) * PARTS)
                yp = ypool.tile([PARTS, HW_], OUT_DT, tag="y", name="yp")
                for k in range(n_conv):
                    ps = psum.tile([PARTS, CONV_N], F32, tag="ps", bufs=4,
                                   name="ps")
                    # (j, m) order: same diag weights for both sub-chunks
                    for j in range(P):
                        for m in range(mm_per_conv):
                            mv = slice(k * CONV_N + m * MM_N,
                                       k * CONV_N + (m + 1) * MM_N)
                            nc.tensor.matmul(
                                ps[:, m * MM_N : (m + 1) * MM_N],
                                Wd[:, 3 * i + j, :],
                                v16[:, j, mv],
                                start=(j == 0), stop=(j == 2),
                            )
                    dst = yp[:, k * CONV_N : (k + 1) * CONV_N]
                    if conv_engine[k] == "a":
                        nc.scalar.copy(dst, ps[:])
                    else:
                        nc.vector.tensor_copy(dst, ps[:])
                # out-DMA on gpsimd's SWDGE queue: keeps SP free for loads
                # and never head-of-line blocks ACT/DVE compute
                nc.gpsimd.dma_start(y_d[cs, i, :], yp[:])

            def alloc_v():
                v32 = vpool.tile([PARTS, P, HW_], F32, tag="v32", name="v32")
                v16 = vpool.tile([PARTS, P, HW_], F16, tag="v16", name="v16")
                return v32, v16

            def alloc_E():
                E = smalls.tile([PARTS, 9], F32, name="E")
                Eh = smalls.tile([PARTS, 4], F32, name="Eh")
                return E, Eh

            # --- prologue: group 0 load + convert + gram + weights ---
            v32_cur, v_cur = alloc_v()
            for i in range(P):
                for h in range(2):
                    load_half(v32_cur, groups[0], i, h)
            E, Eh = alloc_E()
            for i in range(P):
                for h in range(2):
                    convert_half(v32_cur, v_cur, i, h)
                diag(v_cur, E, i)
            cross(v_cur, E[:, 1:2], 0, 1)
            cross(v_cur, Eh[:, 0:1], 1, 2, h=0)
            cross(v_cur, Eh[:, 1:2], 1, 2, h=1)
            cross(v_cur, Eh[:, 2:3], 0, 2, h=0)
            cross(v_cur, Eh[:, 3:4], 0, 2, h=1)
            Wd_cur = softmax_wd(E, Eh)

            for t in range(T):
                g = groups[t]
                have_next = t + 1 < T
                if have_next:
                    gn = groups[t + 1]
                    v32_next, v_next = alloc_v()
                    for i in range(P):
                        for h in range(2):
                            load_half(v32_next, gn, i, h)
                    E, Eh = alloc_E()

                # path 0 of g: drain chunk 0 on DVE (early, before its gram
                # work), the rest on ACT
                path_compute(g, v_cur, Wd_cur, 0, "daaa")
                if have_next:
                    convert_half(v32_next, v_next, 0, 0)   # ready ~2.9us
                    convert_half(v32_next, v_next, 0, 1)   # ready ~5.9us
                    diag(v_next, E, 0)
                path_compute(g, v_cur, Wd_cur, 1, "aaaa")
                if have_next:
                    convert_half(v32_next, v_next, 1, 0)
                    convert_half(v32_next, v_next, 1, 1)   # ready ~11.8us
                    cross(v_next, E[:, 1:2], 0, 1)         # DVE
                    diag(v_next, E, 1)                     # ACT
                path_compute(g, v_cur, Wd_cur, 2, "aaaa")
                if have_next:
                    # halves of the path-2-gated crosses: first halves ready
                    # at ~14.8us, second halves at ~17.6us
                    convert_half(v32_next, v_next, 2, 0)
                    cross(v_next, Eh[:, 0:1], 1, 2, h=0)
                    cross(v_next, Eh[:, 2:3], 0, 2, h=0)
                    convert_half(v32_next, v_next, 2, 1)
                    diag(v_next, E, 2)                     # ACT
                    cross(v_next, Eh[:, 1:2], 1, 2, h=1)
                    cross(v_next, Eh[:, 3:4], 0, 2, h=1)
                    Wd_cur = softmax_wd(E, Eh)
                    v_cur = v_next
                    v32_cur = v32_next

    nc.compile()
    return nc


_NC_CACHE = {}


def _get_nc(C_=C, HW_=HW):
    key = (C_, HW_)
    if key not in _NC_CACHE:
        _NC_CACHE[key] = build_nc(C_, HW_)
    return _NC_CACHE[key]


def run_full(x: np.ndarray, gamma: np.ndarray, **runner_kwargs):
    """Run on all 8 cores; returns the raw BassKernelResults."""
    x = np.asarray(x, dtype=np.float32)
    gamma = np.asarray(gamma, dtype=np.float32)
    assert x.shape == (B, C, P, H, W), x.shape

    nc = _get_nc()
    in_maps = [
        {
            "x": np.ascontiguousarray(x[k]).reshape(C, P, HW),
            "gamma": gamma.reshape(1, 1),
        }
        for k in range(N_CORES)
    ]
    return run_bass_kernel_spmd(
        nc, in_maps, core_ids=list(range(N_CORES)), **runner_kwargs
    )


def kernel(x: np.ndarray, gamma: np.ndarray) -> np.ndarray:
    res = run_full(x, gamma)
    y = np.stack([res.results[k]["y"] for k in range(N_CORES)])
    return y.reshape(B, C, P, H, W).astype(np.float32)
